# revision 1
# baseline (speedup 1.0000x reference)
"""Causal multi-head attention layer (B=2, T=2048, C=2048, H=16) on 8 TRN2
NeuronCores — v3: bf16 + weight-stationary matmul ordering + redundant
LDWEIGHTS elimination.  (v1 fp32r: 487675ns; v2: 454541ns; v3: 418466ns.)

Sharding: data-parallel over batch (2 groups of 4 cores), tensor-parallel over
heads within a group (4 heads/core, Megatron column-split of w_attn and
row-split of w_proj).  Each core computes a partial projection output in
transposed layout yT = (O_heads @ w_proj[:, cols].T).T; the host transposes,
sums the 4 partials per batch element and adds b_proj.

v2 changes vs v1 (all matmuls bf16; psum stays fp32):
  - bf16 operands: FWL halves the LDWEIGHTS cost, x/w/y DMA halves, and
    narrow matmuls run at 1 cyc/row (no fp32r N>=256 restriction), so causal
    skipping is uncapped (up to 384 of 512 masked columns skipped).
  - weight-stationary loop orders to amortize LDWEIGHTS:
      q/k: (pass, u, cc-outer, strip-inner) -> one w-slice load per 4 MMs,
      full-K psum accumulation (16 chunks) -> one DVE flush per (u, strip);
      attention: strips processed in pairs -> kt/vt/ones loads shared by the
      2 strips' S/PV/sum matmuls;
      proj: transposed output (yT), (cout-block, hp-outer, strip-inner) ->
      one wp-slice load per 4 MMs.
  - x, w all resident in SBUF in bf16 (xc 64KB/p + w 32KB/p + qkvo 64KB/p).
"""

import numpy as np
import ml_dtypes

import concourse.bacc as bacc
import concourse.tile as tile
from concourse import mybir
from concourse.bass_utils import run_bass_kernel_spmd

F32 = mybir.dt.float32
BF16 = mybir.dt.bfloat16

B, T, C, H = 2, 2048, 2048, 16
HD = C // H            # 128
HLOC = 4               # heads per core
NCORES = 8
NSTRIP = T // 512      # 4 t-strips
NCH = C // 128         # 16 contraction chunks
SCALE = 1.0 / float(np.sqrt(HD))
NEG = -1.0e30

_cache = {}


def _build_nc(reps=1):
    nc = bacc.Bacc("TRN2", debug=False)

    xt = nc.dram_tensor("xt", [C, T], BF16, kind="ExternalInput")      # x[b].T
    wqkv = nc.dram_tensor("wqkv", [C, 3 * 512], BF16, kind="ExternalInput")
    wp = nc.dram_tensor("wp", [512, C], BF16, kind="ExternalInput")
    maskneg = nc.dram_tensor("maskneg", [128, 128], F32, kind="ExternalInput")
    ones_in = nc.dram_tensor("ones_in", [128, 128], BF16, kind="ExternalInput")
    yt = nc.dram_tensor("yt", [C, T], BF16, kind="ExternalOutput")

    with tile.TileContext(nc) as tc:
        with (
            tc.tile_pool(name="persist", bufs=1) as persist,
            tc.tile_pool(name="work", bufs=2) as work,
            tc.tile_pool(name="psum", bufs=8, space="PSUM") as psum,
        ):
            qt = persist.tile([128, HLOC * T], BF16, tag="qt")
            kt = persist.tile([128, HLOC * T], BF16, tag="kt")
            vt = persist.tile([128, HLOC * T], BF16, tag="vt")
            ot = persist.tile([128, HLOC * T], BF16, tag="ot")
            tri = persist.tile([128, 128], F32, tag="tri")
            ones = persist.tile([128, 128], BF16, tag="ones")
            nc.sync.dma_start(out=tri, in_=maskneg[:, :])
            nc.sync.dma_start(out=ones, in_=ones_in[:, :])

            if reps > 1:
                loop_ctx = tc.For_i(
                    0, reps, 1,
                    hint_engines=(mybir.EngineType.PE,
                                  mybir.EngineType.DVE,
                                  mybir.EngineType.Activation,
                                  mybir.EngineType.SP,
                                  mybir.EngineType.Pool))
                loop_ctx.__enter__()

            # ---- DMA: x chunks + q/k weights interleaved, then v weights ----
            xcs = {}
            wts = {}
            for cc in range(NCH):
                for pas in range(2):       # w_q, w_k chunk for this cc
                    wt = work.tile([128, 512], BF16, tag="wch", bufs=32,
                                   name=f"w_{pas}_{cc}")
                    nc.sync.dma_start(
                        out=wt, in_=wqkv[128 * cc:128 * (cc + 1),
                                         512 * pas:512 * (pas + 1)])
                    wts[(pas, cc)] = wt
                for s in range(NSTRIP):
                    xc = work.tile([128, 512], BF16, tag="xc", bufs=64,
                                   name=f"xc_{cc}_{s}")
                    nc.sync.dma_start(
                        out=xc, in_=xt[128 * cc:128 * (cc + 1),
                                       512 * s:512 * s + 512])
                    xcs[(cc, s)] = xc
            for cc in range(NCH):
                wt = work.tile([128, 512], BF16, tag="wch", bufs=32,
                               name=f"w_2_{cc}")
                nc.sync.dma_start(
                    out=wt, in_=wqkv[128 * cc:128 * (cc + 1), 1024:1536])
                wts[(2, cc)] = wt

            # ---- q/k: weight-stationary, full-K psum accumulation ----
            for pas in range(2):            # 0=q, 1=k
                dst = qt if pas == 0 else kt
                for u in range(4):          # head = output 128-block
                    accs = [psum.tile([128, 512], F32, tag="ps", bufs=8,
                                      name=f"acc_{pas}_{u}_{s}")
                            for s in range(NSTRIP)]
                    for cc in range(NCH):
                        w_u = wts[(pas, cc)][:, 128 * u:128 * (u + 1)]
                        for s in range(NSTRIP):
                            nc.tensor.matmul(
                                accs[s], lhsT=w_u, rhs=xcs[(cc, s)],
                                start=(cc == 0), stop=(cc == NCH - 1))
                    for s in range(NSTRIP):
                        nc.vector.tensor_copy(
                            dst[:, T * u + 512 * s:T * u + 512 * (s + 1)],
                            accs[s])

            # ---- v: [tokens, vchan] orientation (x slices stationary) ----
            for s in range(NSTRIP):
                for u4 in range(4):         # token 128-block within strip
                    j = 4 * s + u4
                    acc = psum.tile([128, 512], F32, tag="ps", bufs=8,
                                    name=f"accv_{j}")
                    for cc in range(NCH):
                        nc.tensor.matmul(
                            acc,
                            lhsT=xcs[(cc, s)][:, 128 * u4:128 * (u4 + 1)],
                            rhs=wts[(2, cc)],
                            start=(cc == 0), stop=(cc == NCH - 1))
                    nc.vector.tensor_copy(vt[:, 512 * j:512 * (j + 1)], acc)

            # ---- proj weights: DMA early so they arrive during attention ----
            wpt = {}
            for hp in range(HLOC):
                for cs in range(4):
                    wt = work.tile([128, 512], BF16, tag="xc", bufs=64,
                                   name=f"wpt_{hp}_{cs}")
                    nc.sync.dma_start(
                        out=wt, in_=wp[128 * hp:128 * (hp + 1),
                                       512 * cs:512 * (cs + 1)])
                    wpt[(hp, cs)] = wt

            # ---- attention: strip pairs, kt/vt loads shared across pair ----
            for sp in range(NSTRIP // 2):
                strips = (2 * sp, 2 * sp + 1)
                for h in range(HLOC):
                    otp = {s2: psum.tile([128, 512], F32, tag="ps", bufs=8,
                                         name=f"otp_{sp}_{h}_{s2}")
                           for s2 in range(2)}
                    sump = {s2: psum.tile([128, 512], F32, tag="ps", bufs=8,
                                          name=f"sump_{sp}_{h}_{s2}")
                            for s2 in range(2)}
                    nj = 4 * strips[1] + 4   # chunks for the later strip

                    def emit_s(j):
                        """S matmul + mask + exp for both strips of chunk j."""
                        pts = {}
                        for s2 in range(2):
                            s = strips[s2]
                            if j >= 4 * (s + 1):
                                continue     # fully masked for this strip
                            d = j - 4 * s    # >=0 on diagonal chunks
                            o = min(128 * d, 384) if d > 0 else 0
                            t0 = 512 * s
                            stp = psum.tile([128, 512], F32, tag="ps", bufs=8,
                                            name=f"stp_{sp}_{h}_{j}_{s2}")
                            nc.tensor.matmul(
                                stp[:, o:],
                                lhsT=kt[:, T * h + 128 * j:T * h + 128 * (j + 1)],
                                rhs=qt[:, T * h + t0 + o:T * h + t0 + 512],
                                start=True, stop=True)
                            pt = work.tile([128, 512], BF16, tag="pt", bufs=6,
                                           name=f"pt_{sp}_{h}_{j}_{s2}")
                            if 0 <= d:
                                nc.vector.tensor_add(
                                    stp[:, 128 * d:128 * (d + 1)],
                                    stp[:, 128 * d:128 * (d + 1)],
                                    tri)
                            nc.scalar.activation(
                                pt[:, o:], stp[:, o:],
                                mybir.ActivationFunctionType.Exp,
                                scale=SCALE)
                            pts[s2] = (pt, o)
                        return pts

                    def emit_pv(j, pts):
                        for s2, (pt, o) in pts.items():
                            s = strips[s2]
                            nc.tensor.matmul(
                                otp[s2][:, o:],
                                lhsT=vt[:, 512 * j + 128 * h:
                                        512 * j + 128 * (h + 1)],
                                rhs=pt[:, o:],
                                start=(j == 0), stop=(j == 4 * s + 3))
                        for s2, (pt, o) in pts.items():
                            s = strips[s2]
                            nc.tensor.matmul(
                                sump[s2][:, o:], lhsT=ones, rhs=pt[:, o:],
                                start=(j == 0), stop=(j == 4 * s + 3))

                    prev = None
                    for j in range(nj):
                        cur = emit_s(j)
                        if prev is not None:
                            emit_pv(j - 1, prev)
                        prev = cur
                    emit_pv(nj - 1, prev)

                    for s2 in range(2):
                        t0 = 512 * strips[s2]
                        rin = work.tile([128, 512], F32, tag="rin", bufs=4,
                                        name=f"r_{sp}_{h}_{s2}")
                        nc.vector.reciprocal(rin, sump[s2])
                        nc.vector.tensor_mul(
                            ot[:, T * h + t0:T * h + t0 + 512],
                            otp[s2], rin)

            # ---- projection  yT[cout, t] = wp-slices.T x ot-strips ----
            for cb in range(16):            # cout 128-blocks
                hp0, cs = 0, cb // 4
                cbo = 128 * (cb % 4)
                ypps = [psum.tile([128, 512], F32, tag="ps", bufs=8,
                                  name=f"yp_{cb}_{s}") for s in range(NSTRIP)]
                for hp in range(HLOC):
                    w_cb = wpt[(hp, cs)][:, cbo:cbo + 128]
                    for s in range(NSTRIP):
                        nc.tensor.matmul(
                            ypps[s], lhsT=w_cb,
                            rhs=ot[:, T * hp + 512 * s:T * hp + 512 * (s + 1)],
                            start=(hp == 0), stop=(hp == HLOC - 1))
                for s in range(NSTRIP):
                    ysb = work.tile([128, 512], BF16, tag="ysb", bufs=6,
                                    name=f"ysb_{cb}_{s}")
                    nc.vector.tensor_copy(ysb, ypps[s])
                    nc.sync.dma_start(
                        out=yt[128 * cb:128 * (cb + 1),
                               512 * s:512 * (s + 1)],
                        in_=ysb)

            if reps > 1:
                loop_ctx.__exit__(None, None, None)

    nc.compile()
    _strip_redundant_ldweights(nc)
    return nc


def _strip_redundant_ldweights(nc):
    """Remove back-to-back InstLdweights that reload the exact weights already
    resident in the PE array (legalization emits one per matmul with no dedup;
    each serialized reload costs ~53-107ns on HW).  Only sync-free loads whose
    (weights AP, perf_mode, tile_position) matches the immediately preceding
    PE weight state are dropped; weight state is conservatively reset at block
    boundaries and on any non-matmul PE instruction."""

    def ap_sig(ap):
        try:
            return ap.to_json()
        except Exception:
            return repr(ap)

    for blk in nc.m.functions[0].blocks:
        cur = None
        keep = []
        changed = False
        for inst in blk.instructions:
            if getattr(inst, "engine", None) != mybir.EngineType.PE:
                keep.append(inst)
                continue
            nm = inst.__class__.__name__
            if nm == "InstLdweights":
                sig = (ap_sig(inst.ins[0]), getattr(inst, "perf_mode", None),
                       getattr(inst, "tile_position", None))
                si = inst.sync_info
                sync_free = not (si and (si.on_wait or si.on_update))
                if sig == cur and sync_free:
                    changed = True
                    continue
                cur = sig
            elif nm != "InstMatmult":
                cur = None
            keep.append(inst)
        if changed:
            blk.instructions = keep


def _host_inputs(x, w_attn, w_proj):
    """Per-core input dicts."""
    x = np.asarray(x, dtype=np.float32)
    w_attn = np.asarray(w_attn, dtype=np.float32)
    w_proj = np.asarray(w_proj, dtype=np.float32)

    p = np.arange(128)[:, None]
    f = np.arange(128)[None, :]
    maskneg = np.where(p <= f, 0.0, NEG).astype(np.float32)
    ones = np.ones((128, 128), dtype=ml_dtypes.bfloat16)

    in_maps = []
    for core in range(NCORES):
        b, g = divmod(core, 4)
        r0 = 512 * g
        wq = w_attn[r0:r0 + 512, :]            # [512, C]
        wk = w_attn[C + r0:C + r0 + 512, :]
        wv = w_attn[2 * C + r0:2 * C + r0 + 512, :]
        wqkv = np.ascontiguousarray(
            np.concatenate([wq.T, wk.T, wv.T], axis=1)).astype(
                ml_dtypes.bfloat16)            # [C, 1536]
        wpm = np.ascontiguousarray(w_proj[:, r0:r0 + 512].T).astype(
            ml_dtypes.bfloat16)                # [512, C]
        in_maps.append({
            "xt": np.ascontiguousarray(x[b].T).astype(ml_dtypes.bfloat16),
            "wqkv": wqkv,
            "wp": wpm,
            "maskneg": maskneg,
            "ones_in": ones,
        })
    return in_maps


def kernel(x, w_attn, w_proj, b_proj):
    if "nc" not in _cache:
        _cache["nc"] = _build_nc()
    nc = _cache["nc"]

    in_maps = _host_inputs(x, w_attn, w_proj)
    res = run_bass_kernel_spmd(nc, in_maps, core_ids=list(range(NCORES)))
    _cache["last_result"] = res
    if res.exec_time_ns is not None:
        print(f"HW exec time: {res.exec_time_ns} ns")

    b_proj = np.asarray(b_proj, dtype=np.float32)
    out = np.empty((B, T, C), dtype=np.float32)
    for b in range(B):
        acc = res.results[4 * b]["yt"].astype(np.float32)
        for g in range(1, 4):
            acc = acc + res.results[4 * b + g]["yt"].astype(np.float32)
        out[b] = acc.T + b_proj[None, :]
    return out

